# revision 16
# baseline (speedup 1.0000x reference)
"""Trainium2 Bass kernel for nn_Att_cnn2d (pairwise additive attention ->
2x2 conv -> MLP head), distributed across 8 NeuronCores.

Sharding: attention is data-parallel over batch; conv+Linear1 are
model-parallel (each core owns 8 of 64 conv filters = 1/8 of Linear1's
input features); an AllGather exchanges attention outputs and a
ReduceScatter sums Linear1 partials; the small MLP head is data-parallel.
"""
import os
import sys

sys.path.insert(0, "/opt/trn_rl_repo")
import numpy as np


def _install_ntff_shim():
    """Provide antenv.axon_hooks if the image lacks it (needed only for
    trace=True profiling; harmless otherwise)."""
    try:
        import antenv.axon_hooks  # noqa: F401
        return
    except ImportError:
        pass
    import contextlib
    import ctypes
    import types

    so_path = "/opt/axon/libaxon_pjrt.so"
    _hook = [None]

    def _make_hook():
        if not os.path.exists(so_path):
            return None
        lib = ctypes.CDLL(so_path)
        if not hasattr(lib, "axon_start_nrt_profile"):
            return None
        lib.axon_start_nrt_profile.argtypes = [
            ctypes.POINTER(ctypes.c_int64), ctypes.c_size_t]
        lib.axon_start_nrt_profile.restype = ctypes.c_int64
        lib.axon_stop_nrt_profile.argtypes = [ctypes.c_char_p]
        lib.axon_stop_nrt_profile.restype = ctypes.c_int64

        @contextlib.contextmanager
        def hook(output_dir, device_ids):
            import jax
            jax.devices()
            if device_ids:
                ids = (ctypes.c_int64 * len(device_ids))(*device_ids)
                rc = lib.axon_start_nrt_profile(ids, len(device_ids))
            else:
                rc = lib.axon_start_nrt_profile(None, 0)
            if rc != 0:
                raise RuntimeError(f"axon_start_nrt_profile rc={rc}")
            try:
                yield
            finally:
                n = lib.axon_stop_nrt_profile(str(output_dir).encode())
                print(f"ntff profile: {n} file(s) -> {output_dir}",
                      file=sys.stderr)

        return hook

    mod = types.ModuleType("antenv.axon_hooks")
    mod.get_axon_ntff_profile_hook = lambda: _hook[0]
    mod.set_axon_ntff_profile_hook = lambda h: _hook.__setitem__(0, h)
    _hook[0] = _make_hook()
    sys.modules["antenv.axon_hooks"] = mod


_install_ntff_shim()

# model dims (fixed by the problem)
T, F, H = 32, 64, 64
NFILT, R, C, FB = 64, 31, 127, 128
O1, NCATE = 256, 24
FLAT = NFILT * R * C

LAST_EXEC_NS = None
LAST_RESULT = None
_CACHE = {}


def _sections(nf_loc):
    """Split the 31 conv output rows into sections with nf_loc*rows <= 128."""
    sz = max(1, 128 // nf_loc)
    secs = []
    r0 = 0
    while r0 < R:
        nr = min(sz, R - r0)
        secs.append((r0, nr))
        r0 += nr
    return secs


def _build(n_cores, b_loc, phase="full"):
    from concourse import mybir, tile
    from concourse.mybir import dt
    import concourse.bacc as bacc

    nloc = b_loc * T
    nblk = b_loc // 4
    bglob = b_loc * n_cores
    SG = b_loc                    # phase-C sample group == one core's batch
    nchunk = min(64, SG)          # samples per inpR buffer
    nf_loc = NFILT // n_cores
    secs = _sections(nf_loc)
    nsec = len(secs)

    nc = bacc.Bacc("TRN2", target_bir_lowering=False, debug=False,
                   num_devices=n_cores)
    dtb, dtf = dt.bfloat16, dt.float32

    def din(name, shape, d):
        return nc.dram_tensor(name, shape, d, kind="ExternalInput")

    xt = din("xt", [F, nloc], dtb)
    x = din("x", [nloc, F], dtf)
    wu = din("wu", [F, 2 * H], dtb)
    v128 = din("v128", [128, H], dtb)
    ndmask = din("ndmask", [128, T], dtf)
    convst = din("convst", [64, nsec * 128], dtb)
    convb = din("convb", [128, nsec], dtf)
    w1c = din("w1c", [128, nsec * C * O1], dtb)
    w1e = din("w1e", [NCATE, O1], dtb)
    catev = din("catev", [NCATE, bglob], dtb)
    b1rep = din("b1rep", [b_loc, O1], dtf)
    w2t2 = din("w2t2", [128, 2 * 32], dtb)
    b2c = din("b2c", [32, 1], dtf)
    w3t = din("w3t", [32, 2], dtb)
    b3c = din("b3c", [2, 1], dtf)
    idn = din("idn", [128, 128], dtb)
    out = nc.dram_tensor("out", [b_loc, 2], dtf, kind="ExternalOutput")

    AF = mybir.ActivationFunctionType
    AL = mybir.AluOpType
    rg = [list(range(n_cores))]

    with tile.TileContext(nc) as tc:
        with tc.tile_pool(name="res", bufs=1) as res, \
             tc.tile_pool(name="dram", bufs=1, space="DRAM") as dpool:
            # resident tensors
            w1c_sb = res.tile([128, nsec * C * O1], dtb)
            nc.sync.dma_start(w1c_sb[:], w1c[:, :])
            xt_sb = res.tile([F, nloc], dtb)
            nc.sync.dma_start(xt_sb[:], xt[:, :])
            wu_sb = res.tile([F, 2 * H], dtb)
            nc.sync.dma_start(wu_sb[:], wu[:, :])
            v_sb = res.tile([128, H], dtb)
            nc.sync.dma_start(v_sb[:], v128[:, :])
            ndm_sb = res.tile([128, T], dtf)
            nc.sync.dma_start(ndm_sb[:], ndmask[:, :])
            cst_sb = res.tile([64, nsec * 128], dtb)
            nc.sync.dma_start(cst_sb[:], convst[:, :])
            cb_sb = res.tile([128, nsec], dtf)
            nc.sync.dma_start(cb_sb[:], convb[:, :])
            w1e_sb = res.tile([NCATE, O1], dtb)
            nc.sync.dma_start(w1e_sb[:], w1e[:, :])
            cate_sb = res.tile([NCATE, bglob], dtb)
            nc.sync.dma_start(cate_sb[:], catev[:, :])
            b1_sb = res.tile([b_loc, O1], dtf)
            nc.sync.dma_start(b1_sb[:], b1rep[:, :])
            w2_sb = res.tile([128, 2 * 32], dtb)
            nc.sync.dma_start(w2_sb[:], w2t2[:, :])
            b2_sb = res.tile([32, 1], dtf)
            nc.sync.dma_start(b2_sb[:], b2c[:, :])
            w3_sb = res.tile([32, 2], dtb)
            nc.sync.dma_start(w3_sb[:], w3t[:, :])
            b3_sb = res.tile([2, 1], dtf)
            nc.sync.dma_start(b3_sb[:], b3c[:, :])
            idn_sb = res.tile([128, 128], dtb)
            nc.sync.dma_start(idn_sb[:], idn[:, :])

            xu_dram = dpool.tile([nloc, H], dtb)
            ag_in = dpool.tile([T, b_loc * FB], dtb)
            ag_out = dpool.tile([n_cores * T, b_loc * FB], dtb)
            rs_in = dpool.tile([bglob, O1], dtf)
            rs_out = dpool.tile([b_loc, O1], dtf)

            # ---------------- phase A: attention ----------------
            with tc.tile_pool(name="attn", bufs=3) as apool, \
                 tc.tile_pool(name="attn1", bufs=2) as apool1, \
                 tc.tile_pool(name="attps", bufs=2, space="PSUM") as aps:
                for g in range(nblk):
                    tok = slice(128 * g, 128 * g + 128)
                    x_t = apool1.tile([128, F], dtf, tag="x_t")
                    nc.sync.dma_start(x_t[:], x[tok, :])
                    wxu = aps.tile([128, 2 * H], dtf, tag="wxu")
                    nc.tensor.matmul(wxu[:, :], xt_sb[:, tok], wu_sb[:, :],
                                     start=True, stop=True)
                    xw_sb = apool1.tile([128, H], dtb, tag="xw")
                    nc.scalar.copy(xw_sb[:], wxu[:, 0:H])
                    xu_sb = apool1.tile([128, H], dtb, tag="xu")
                    nc.scalar.copy(xu_sb[:], wxu[:, H:2 * H])
                    nc.sync.dma_start(xu_dram[tok, :], xu_sb[:])
                    xud = apool.tile([128, T * H], dtb, tag="xud")
                    for b in range(4):
                        srcb = xu_dram[128 * g + 32 * b:128 * g + 32 * b + 32, :] \
                            .rearrange("(i r) h -> i (r h)", i=1) \
                            .broadcast_to([32, T * H])
                        nc.sync.dma_start(xud[32 * b:32 * b + 32, :], srcb)
                    arg = apool.tile([128, T * H], dtb, tag="arg")
                    xw_b = xw_sb[:, :].rearrange("p (j h) -> p j h", j=1) \
                        .broadcast_to([128, T, H])
                    nc.vector.tensor_tensor(
                        arg[:, :].rearrange("p (j h) -> p j h", j=T),
                        xw_b,
                        xud[:, :].rearrange("p (j h) -> p j h", j=T),
                        op=AL.add)
                    t_t = apool.tile([128, T * H], dtb, tag="t_t")
                    nc.scalar.activation(t_t[:], arg[:], AF.Tanh)
                    tv = apool.tile([128, T * H], dtb, tag="tv")
                    v_b = v_sb[:, :].rearrange("p (j h) -> p j h", j=1) \
                        .broadcast_to([128, T, H])
                    nc.vector.tensor_tensor(
                        tv[:, :].rearrange("p (j h) -> p j h", j=T),
                        t_t[:, :].rearrange("p (j h) -> p j h", j=T),
                        v_b, op=AL.mult)
                    e_sb = apool1.tile([128, T], dtf, tag="e")
                    nc.vector.tensor_reduce(
                        e_sb[:], tv[:, :].rearrange("p (j h) -> p j h", j=T),
                        axis=mybir.AxisListType.X, op=AL.add)
                    nmax = apool1.tile([128, 1], dtf, tag="nmax")
                    nc.vector.tensor_reduce(nmax[:], e_sb[:],
                                            axis=mybir.AxisListType.X,
                                            op=AL.max, negate=True)
                    expE = apool1.tile([128, T], dtf, tag="expE")
                    sums = apool1.tile([128, 1], dtf, tag="sums")
                    nc.scalar.activation(expE[:], e_sb[:], AF.Exp,
                                         bias=nmax[:], scale=1.0,
                                         accum_out=sums[:])
                    rinv = apool1.tile([128, 1], dtf, tag="rinv")
                    nc.vector.reciprocal(rinv[:], sums[:])
                    dtmp = apool1.tile([128, T], dtf, tag="dtmp")
                    nc.vector.tensor_tensor(dtmp[:], expE[:], ndm_sb[:],
                                            op=AL.mult)
                    ndiag = apool1.tile([128, 1], dtf, tag="ndiag")
                    nc.vector.tensor_reduce(ndiag[:], dtmp[:],
                                            axis=mybir.AxisListType.X,
                                            op=AL.add)
                    expT = apool1.tile([128, T], dtf, tag="expT")
                    nc.vector.transpose(expT[:], expE[:])
                    cu = aps.tile([128, F], dtf, tag="cu")
                    for b in range(4):
                        sb = slice(32 * b, 32 * b + 32)
                        nc.tensor.matmul(cu[sb, :], expT[sb, :], x_t[sb, :],
                                         start=True, stop=True,
                                         tile_position=(32 * b, 32 * b))
                    inp = apool1.tile([128, FB], dtb, tag="inp")
                    nc.vector.tensor_copy(inp[:, 0:F], x_t[:])
                    tmpc = apool1.tile([128, F], dtf, tag="tmpc")
                    nc.vector.scalar_tensor_tensor(tmpc[:], x_t[:], ndiag[:],
                                                   cu[:], op0=AL.mult,
                                                   op1=AL.add)
                    nc.vector.tensor_scalar(inp[:, F:FB], tmpc[:], rinv[:],
                                            None, op0=AL.mult)
                    for b in range(4):
                        s = 4 * g + b
                        nc.sync.dma_start(ag_in[:, FB * s:FB * (s + 1)],
                                          inp[32 * b:32 * b + 32, :])

            def _dbg_out(src_ap):
                with tc.tile_pool(name="dbg", bufs=1) as dbg:
                    tb = dbg.tile([2, b_loc], dtb)
                    nc.sync.dma_start(tb[:], src_ap)
                    t3 = dbg.tile([2, b_loc], dtf)
                    nc.vector.tensor_copy(t3[:], tb[:])
                    nc.sync.dma_start(out.rearrange("s c -> c s"), t3[:])

            do_ag = phase in ("AG", "C", "C1", "C2", "RS", "full")
            do_c = phase in ("C", "C1", "C1b", "C2", "RS", "full")
            do_conv = phase != "C2"
            do_l1 = phase not in ("C1", "C1b")
            do_rs = phase in ("RS", "full")
            do_head = phase == "full"
            if phase == "A":
                _dbg_out(ag_in[0:2, 0:b_loc])

            # ---------------- AllGather ----------------
            if do_ag:
                nc.gpsimd.collective_compute(
                    "AllGather", AL.bypass, ins=[ag_in[:, :]],
                    outs=[ag_out[:, :]], replica_groups=rg)
            if phase == "AG":
                _dbg_out(ag_out[0:2, 0:b_loc])

            # ---------------- phase C: conv + Linear1 ----------------
            with tc.tile_pool(name="convp", bufs=1) as cpool, \
                 tc.tile_pool(name="convp2", bufs=2) as cpool2, \
                 tc.tile_pool(name="convps", bufs=2, space="PSUM") as cps:
                w1c4 = w1c_sb[:, :].rearrange("p (z c o) -> p z c o", z=nsec,
                                              c=C)
                for g2 in range(n_cores if do_c else 0):
                    o1_ps = cps.tile([SG, O1], dtf, tag="o1")
                    for isec, (r0, nr) in enumerate(secs):
                        Kr = nf_loc * nr
                        flatT = cpool.tile([128, C * SG], dtb, tag="flatT")
                        if not do_conv:
                            nc.vector.memset(flatT[:], 0.25)
                        for ch in range(SG // nchunk if do_conv else 0):
                            inpRa = cpool2.tile([64, nchunk * FB], dtb,
                                                tag="inpR")
                            src3 = ag_out[T * g2:T * g2 + T, :] \
                                .rearrange("p (s c) -> p s c", s=SG)[
                                    :, ch * nchunk:(ch + 1) * nchunk, :]
                            dst3 = inpRa[:, :].rearrange(
                                "p (s c) -> p s c", s=nchunk)
                            nc.sync.dma_start(dst3[0:32, :, :], src3)
                            nc.sync.dma_start(dst3[32:64, :, 0:C],
                                              src3[:, :, 1:FB])
                            inpR3 = inpRa[:, :].rearrange(
                                "p (s c) -> p s c", s=nchunk)
                            for g3 in range(nchunk // 4):
                                cp = cps.tile([Kr, 4 * C], dtf, tag="cp")
                                nc.tensor.matmul(
                                    cp[:, :],
                                    cst_sb[0:64,
                                           128 * isec:128 * isec + Kr],
                                    inpR3[0:64, 4 * g3:4 * g3 + 4, 0:C],
                                    start=True, stop=True)
                                s0 = ch * nchunk + 4 * g3
                                dst = flatT[0:Kr, :].rearrange(
                                    "p (c s) -> p s c", c=C)[:, s0:s0 + 4, :]
                                nc.scalar.activation(
                                    dst,
                                    cp[:, :].rearrange("p (s c) -> p s c",
                                                       s=4),
                                    AF.Relu, bias=cb_sb[0:Kr, isec:isec + 1])
                        flatT3 = flatT[:, :].rearrange("p (c s) -> p c s",
                                                       c=C)
                        for c in range(C if do_l1 else 0):
                            nc.tensor.matmul(
                                o1_ps[:, :], flatT3[0:Kr, c, :],
                                w1c4[0:Kr, isec, c, :],
                                start=(isec == 0 and c == 0), stop=False)
                    nc.tensor.matmul(o1_ps[:, :],
                                     cate_sb[:, SG * g2:SG * (g2 + 1)],
                                     w1e_sb[:, :], start=not do_l1,
                                     stop=True)
                    o1_sb = cpool2.tile([SG, O1], dtf, tag="o1sb")
                    nc.vector.tensor_copy(o1_sb[:], o1_ps[:])
                    nc.sync.dma_start(rs_in[SG * g2:SG * (g2 + 1), :],
                                      o1_sb[:])

            if phase in ("C", "C1", "C1b", "C2"):
                nc.sync.dma_start(out[:, :], rs_in[0:b_loc, 0:2])

            # ---------------- ReduceScatter ----------------
            if do_rs:
                nc.gpsimd.collective_compute(
                    "ReduceScatter", AL.add, ins=[rs_in[:, :]],
                    outs=[rs_out[:, :]], replica_groups=rg)
            if phase == "RS":
                nc.sync.dma_start(out[:, :], rs_out[0:b_loc, 0:2])

            # ---------------- phase E: MLP head ----------------
            if not do_head:
                pass
            else:
              with tc.tile_pool(name="head", bufs=1) as hpool, \
                 tc.tile_pool(name="headps", bufs=1, space="PSUM") as hps:
                rs_sb = hpool.tile([b_loc, O1], dtf)
                nc.sync.dma_start(rs_sb[:], rs_out[:, :])
                h1s = hpool.tile([b_loc, O1], dtf)
                nc.vector.tensor_tensor(h1s[:], rs_sb[:], b1_sb[:], op=AL.add)
                h1r = hpool.tile([b_loc, O1], dtb)
                nc.vector.tensor_relu(h1r[:], h1s[:])
                h1T = hpool.tile([128, 2 * b_loc], dtb)
                for chn in range(2):
                    tp = hps.tile([128, b_loc], dtb, tag="tp")
                    nc.tensor.transpose(tp[:, :],
                                        h1r[:, 128 * chn:128 * chn + 128],
                                        idn_sb[0:b_loc, 0:b_loc])
                    nc.vector.tensor_copy(
                        h1T[:, b_loc * chn:b_loc * (chn + 1)], tp[:, :])
                o2_ps = hps.tile([32, b_loc], dtf, tag="o2")
                for chn in range(2):
                    nc.tensor.matmul(o2_ps[:, :],
                                     w2_sb[:, 32 * chn:32 * chn + 32],
                                     h1T[:, b_loc * chn:b_loc * (chn + 1)],
                                     start=(chn == 0), stop=(chn == 1))
                h2 = hpool.tile([32, b_loc], dtb)
                nc.scalar.activation(h2[:], o2_ps[:], AF.Relu, bias=b2_sb[:])
                o3_ps = hps.tile([2, b_loc], dtf, tag="o3")
                nc.tensor.matmul(o3_ps[:, :], w3_sb[:, :], h2[:, :],
                                 start=True, stop=True)
                o3_sb = hpool.tile([2, b_loc], dtf)
                nc.vector.tensor_scalar(o3_sb[:], o3_ps[:], b3_sb[:], None,
                                        op0=AL.add)
                nc.sync.dma_start(out.rearrange("s c -> c s"), o3_sb[:])

    nc.compile()
    return nc


def _prep(inputs, n_cores, b_loc):
    from concourse.mybir import dt
    bf16 = dt.np(dt.bfloat16)

    def bf(a):
        return np.ascontiguousarray(a, np.float32).astype(bf16)

    bglob = n_cores * b_loc
    X = np.ascontiguousarray(inputs["X_nume"], np.float32)[:bglob]
    Xc = np.asarray(inputs["X_cate"])[:bglob]
    W_att = np.asarray(inputs["W_att"], np.float32)
    U_att = np.asarray(inputs["U_att"], np.float32)
    V_att = np.asarray(inputs["V_att"], np.float32)
    conv_w = np.asarray(inputs["conv_w"], np.float32)
    conv_b = np.asarray(inputs["conv_b"], np.float32)
    emb = [np.asarray(inputs[f"emb{i}"], np.float32) for i in range(3)]
    w1 = np.asarray(inputs["w1"], np.float32)
    b1 = np.asarray(inputs["b1"], np.float32)
    w2 = np.asarray(inputs["w2"], np.float32)
    b2 = np.asarray(inputs["b2"], np.float32)
    w3 = np.asarray(inputs["w3"], np.float32)
    b3 = np.asarray(inputs["b3"], np.float32)

    nf_loc = NFILT // n_cores
    secs = _sections(nf_loc)
    nsec = len(secs)

    shared = {}
    shared["wu"] = bf(np.concatenate([W_att, U_att], axis=1))
    shared["v128"] = bf(np.tile(V_att[:, 0][None, :], (128, 1)))
    ndm = np.zeros((128, T), np.float32)
    ndm[np.arange(128), np.arange(128) % T] = -1.0
    shared["ndmask"] = ndm
    shared["w1e"] = bf(w1[:, FLAT:].T)
    shared["b1rep"] = np.tile(b1[None, :], (b_loc, 1)).astype(np.float32)
    shared["w2t2"] = bf(np.concatenate([w2.T[:128], w2.T[128:]], axis=1))
    shared["b2c"] = b2.reshape(32, 1).astype(np.float32)
    shared["w3t"] = bf(w3.T)
    shared["b3c"] = b3.reshape(2, 1).astype(np.float32)
    shared["idn"] = bf(np.eye(128))

    cate_all = np.concatenate(
        [emb[i][Xc[:, i]] for i in range(3)], axis=1)  # [bglob, 24]
    w1conv = w1[:, :FLAT].reshape(O1, NFILT, R, C)

    percore = []
    for k in range(n_cores):
        d = dict(shared)
        sl = slice(k * b_loc, (k + 1) * b_loc)
        Xk = X[sl].reshape(-1, F)
        d["x"] = np.ascontiguousarray(Xk)
        d["xt"] = bf(Xk.T)
        cv = np.zeros((NCATE, bglob), np.float32)
        cv[:, sl] = cate_all[sl].T
        d["catev"] = bf(cv)

        fs = slice(k * nf_loc, (k + 1) * nf_loc)
        cw = conv_w[fs, 0]      # [nf_loc, 2, 2]
        cb = conv_b[fs]
        cst = np.zeros((64, nsec * 128), np.float32)
        cbias = np.zeros((128, nsec), np.float32)
        w1ck = np.zeros((128, nsec, C, O1), np.float32)
        for isec, (r0, nr) in enumerate(secs):
            for dc in range(2):
                for f in range(nf_loc):
                    for rl in range(nr):
                        for dr in range(2):
                            cst[32 * dc + r0 + rl + dr,
                                128 * isec + nr * f + rl] = cw[f, dr, dc]
            for f in range(nf_loc):
                cbias[nr * f:nr * (f + 1), isec] = cb[f]
                for rl in range(nr):
                    w1ck[nr * f + rl, isec, :, :] = \
                        w1conv[:, k * nf_loc + f, r0 + rl, :].T
        d["convst"] = bf(cst)
        d["convb"] = cbias
        d["w1c"] = bf(w1ck.reshape(128, -1))
        percore.append(d)
    return percore


def kernel(**inputs):
    global LAST_EXEC_NS
    n_cores, b_loc = 8, 128
    key = (n_cores, b_loc)
    if key not in _CACHE:
        _CACHE[key] = _build(n_cores, b_loc)
    nc = _CACHE[key]
    in_maps = _prep(inputs, n_cores, b_loc)
    from concourse.bass_utils import run_bass_kernel_spmd
    res = run_bass_kernel_spmd(nc, in_maps, core_ids=list(range(n_cores)),
                               trace=bool(os.environ.get("KTRACE")))
    LAST_EXEC_NS = res.exec_time_ns
    global LAST_RESULT
    LAST_RESULT = res
    outs = [np.asarray(res.results[i]["out"]) for i in range(n_cores)]
    return np.concatenate(outs, axis=0).astype(np.float32)
